# revision 9
# baseline (speedup 1.0000x reference)
"""Trainium2 Bass kernel for the MDA GNN (3x GAT views + MS-CAM fusion + pair MLP).

Distribution over 8 NeuronCores (single SPMD launch):
  stage 1  h = feat @ W.T (+asrc,+adst cols)  -- sharded by source rows j
  AllGather per view of the [CJ, 904] bf16 block
  stage 2  masked-softmax attention            -- sharded by the 1778 used
           target rows i (223 per core), psum-accumulated over all j
  CAM      channel attention, global BN stats via 2 tiny AllGathers
  output   per-core q,r = sum_x @ (wa|wb) chunks; host does q[a]+r[b]+c
           (the 4-layer pair MLP has no activations -> collapses to one
           1802-dim vector on host)
"""

import numpy as np
import ml_dtypes

import concourse.bass as bass
import concourse.mybir as mybir
import concourse.tile as tile
from concourse import bacc
from concourse.bass_utils import run_bass_kernel_spmd

BF16 = mybir.dt.bfloat16
F32 = mybir.dt.float32
AF = mybir.ActivationFunctionType
MUL = mybir.AluOpType.mult
ADD = mybir.AluOpType.add

NCORES = 8
OUT = 901
HC = OUT + 3          # cols: 0..900 h, 901 asrc, 902 adst, 903 ones/pad
NROWS = 1778
MI = 1784             # padded fused rows (8*223)
CI = MI // NCORES     # 223
NPAIRS = 4096
EPS = 1e-5
CNT = float(NROWS * OUT)

VIEWS = [
    dict(name="drug", N=2060, off=1183),
    dict(name="inc", N=2459, off=1582),
    dict(name="mrna", N=3929, off=3052),
]
for V in VIEWS:
    V["CJ"] = -(-V["N"] // NCORES)          # per-core source rows
    V["JG"] = V["CJ"] * NCORES              # gathered rows
    V["NK"] = -(-V["N"] // 128)             # contraction tiles
    V["KP"] = V["NK"] * 128
    V["NJS"] = -(-V["CJ"] // 128)           # stage-1 j subtiles
    V["NJT"] = -(-V["JG"] // 128)           # stage-2 j tiles

ISUBS = [(0, 128), (128, CI - 128)]         # i subtiles within 223

_CACHE = {}
LAST_RESULTS = None


def _bcast(ap, parts, cols, offset=0):
    """Partition-broadcast AP over a DRAM row."""
    return bass.AP(tensor=ap.tensor, offset=ap.offset + offset,
                   ap=[[0, parts], [1, cols]])


def build_graph():
    nc = bacc.Bacc("TRN2", target_bir_lowering=False, debug=False,
                   enable_asserts=False, num_devices=NCORES)
    ins = {}
    for V in VIEWS:
        n = V["name"]
        ins[f"featT_{n}"] = nc.dram_tensor(f"featT_{n}", [128, V["NK"] * V["CJ"]], BF16, kind="ExternalInput").ap()
        ins[f"featU_{n}"] = nc.dram_tensor(f"featU_{n}", [128, V["NK"] * CI], BF16, kind="ExternalInput").ap()
        ins[f"Wx_{n}"] = nc.dram_tensor(f"Wx_{n}", [128, V["NK"] * HC], BF16, kind="ExternalInput").ap()
        ins[f"maskT_{n}"] = nc.dram_tensor(f"maskT_{n}", [V["JG"], CI], BF16, kind="ExternalInput").ap()
        ins[f"b_{n}"] = nc.dram_tensor(f"b_{n}", [1, OUT], F32, kind="ExternalInput").ap()
    ins["md"] = nc.dram_tensor("md", [CI, OUT], BF16, kind="ExternalInput").ap()
    ins["validi"] = nc.dram_tensor("validi", [CI, 1], F32, kind="ExternalInput").ap()
    ins["camw"] = nc.dram_tensor("camw", [1, 16], F32, kind="ExternalInput").ap()
    ins["wab"] = nc.dram_tensor("wab", [2, OUT], F32, kind="ExternalInput").ap()
    qr_out = nc.dram_tensor("qr", [CI, 2], F32, kind="ExternalOutput").ap()
    rg = [list(range(NCORES))]

    with tile.TileContext(nc) as tc:
        with (
            tc.tile_pool(name="persist", bufs=1) as per,
            tc.tile_pool(name="stream", bufs=2) as st,
            tc.tile_pool(name="dram", bufs=1, space="DRAM") as dr,
            tc.tile_pool(name="ps_s1", bufs=1, space="PSUM") as ps1,
            tc.tile_pool(name="ps_s2", bufs=1, space="PSUM") as ps2p,
            tc.tile_pool(name="ps_sm", bufs=2, space="PSUM") as pss,
        ):
            # ---- constants / small broadcasts ----
            ones = per.tile([128, 1], F32, tag="ones")
            nc.vector.memset(ones, 1.0)
            epst = per.tile([1, 1], F32, tag="epst")
            nc.vector.memset(epst, EPS)
            camb = per.tile([128, 16], F32, tag="camb")
            nc.sync.dma_start(camb, _bcast(ins["camw"], 128, 16))
            valid, invalid, mdt = {}, {}, {}
            for s, (i0, isz) in enumerate(ISUBS):
                valid[s] = per.tile([128, 1], F32, tag=f"valid{s}", name=f"valid{s}")
                nc.sync.dma_start(valid[s][:isz], ins["validi"][i0:i0 + isz, :])
                invalid[s] = per.tile([128, 1], F32, tag=f"invalid{s}", name=f"invalid{s}")
                nc.vector.tensor_scalar(invalid[s][:isz], valid[s][:isz],
                                        -1.0, 1.0, op0=MUL, op1=ADD)
                mdt[s] = per.tile([128, OUT], BF16, tag=f"mdt{s}", name=f"mdt{s}")
                nc.sync.dma_start(mdt[s][:isz], ins["md"][i0:i0 + isz, :])

            xs = {}       # (chan, isub) -> bf16 [p, OUT] tiles
            xs[(3, 0)], xs[(3, 1)] = mdt[0], mdt[1]
            agouts = {}
            adstbc = {}
            # =================== per-view stage 1 + AG + arow ===========
            for vi, V in enumerate(VIEWS):
                n, CJ, NK, NJS = V["name"], V["CJ"], V["NK"], V["NJS"]
                featT = per.tile([128, NK * CJ], BF16, tag="featT")
                nc.sync.dma_start(featT, ins[f"featT_{n}"][:, :])
                wx = per.tile([128, NK * HC], BF16, tag="wx")
                nc.sync.dma_start(wx, ins[f"Wx_{n}"][:, :])
                featu = per.tile([128, NK * CI], BF16, tag="featu")
                nc.sync.dma_start(featu, ins[f"featU_{n}"][:, :])

                agin = dr.tile([CJ, HC], BF16, tag=f"agin{vi}")
                agout = dr.tile([V["JG"], HC], BF16, tag=f"agout{vi}",
                                addr_space="Shared")
                agouts[vi] = agout

                for js in range(NJS):
                    pj = min(128, CJ - js * 128)
                    h0 = ps1.tile([128, 452], F32, tag="s1ps0")
                    h1 = ps1.tile([128, 452], F32, tag="s1ps1")
                    for kt in range(NK):
                        lhsT = featT[:, kt * CJ + js * 128: kt * CJ + js * 128 + pj]
                        nc.tensor.matmul(h0[:pj], lhsT,
                                         wx[:, kt * HC: kt * HC + 452],
                                         start=(kt == 0), stop=(kt == NK - 1))
                        nc.tensor.matmul(h1[:pj], lhsT,
                                         wx[:, kt * HC + 452: kt * HC + 904],
                                         start=(kt == 0), stop=(kt == NK - 1))
                    s1out = st.tile([128, HC], BF16, tag="s1out")
                    nc.vector.tensor_copy(s1out[:pj, 0:452], h0[:pj])
                    nc.vector.tensor_copy(s1out[:pj, 452:904], h1[:pj])
                    nc.sync.dma_start(agin[js * 128: js * 128 + pj, :], s1out[:pj])

                # adst row for my fused i-chunk: [1, CI] = wdst.T @ featU
                arow = pss.tile([1, CI], F32, tag="small")
                for kt in range(NK):
                    nc.tensor.matmul(arow[:1], wx[:, kt * HC + 902: kt * HC + 903],
                                     featu[:, kt * CI: (kt + 1) * CI],
                                     start=(kt == 0), stop=(kt == NK - 1))
                arow_sb = st.tile([1, CI], F32, tag="arowsb")
                nc.vector.tensor_copy(arow_sb, arow)
                adr = dr.tile([1, CI], F32, tag=f"adr{vi}")
                nc.sync.dma_start(adr, arow_sb)
                abc = per.tile([128, CI], F32, tag=f"adstbc{vi}")
                nc.sync.dma_start(abc, _bcast(adr, 128, CI))
                adstbc[vi] = abc

                nc.gpsimd.collective_compute(
                    "AllGather", mybir.AluOpType.bypass, replica_groups=rg,
                    ins=[agin.opt()], outs=[agout.opt()])

            # =================== per-view stage 2 =======================
            for vi, V in enumerate(VIEWS):
                n, NJT, JG = V["name"], V["NJT"], V["JG"]
                agout = agouts[vi]
                ps2 = [[ps2p.tile([128, 452], F32, tag=f"s2ps{s}{h}",
                                  name=f"s2ps{s}{h}")
                        for h in range(2)] for s in range(2)]
                for jt in range(NJT):
                    pj = min(128, JG - jt * 128)
                    ht = st.tile([128, HC], BF16, tag="ht")
                    nc.sync.dma_start(ht[:pj], agout[jt * 128: jt * 128 + pj, :])
                    nc.vector.memset(ht[:pj, 903:904], 1.0)
                    mt = st.tile([128, CI], BF16, tag="mt")
                    nc.sync.dma_start(mt[:pj], ins[f"maskT_{n}"][jt * 128: jt * 128 + pj, :])
                    asr = st.tile([128, 1], F32, tag="asr")
                    nc.vector.tensor_copy(asr[:pj], ht[:pj, 901:902])
                    et = st.tile([128, CI], F32, tag="et")
                    nc.scalar.activation(et[:pj], adstbc[vi][:pj], AF.Lrelu,
                                         bias=asr[:pj], scale=1.0, alpha=0.2)
                    pt = st.tile([128, CI], BF16, tag="pt")
                    nc.scalar.activation(pt[:pj], et[:pj], AF.Exp)
                    nc.vector.tensor_mul(pt[:pj], pt[:pj], mt[:pj])
                    for s, (i0, isz) in enumerate(ISUBS):
                        for h in range(2):
                            nc.tensor.matmul(
                                ps2[s][h][:isz], pt[:pj, i0:i0 + isz],
                                ht[:pj, h * 452: (h + 1) * 452],
                                start=(jt == 0), stop=(jt == NJT - 1))
                # epilogue: v = relu(out / rowsum + b)
                bbc = per.tile([128, OUT], F32, tag="bbc", bufs=2)
                nc.sync.dma_start(bbc, _bcast(ins[f"b_{n}"], 128, OUT))
                for s, (i0, isz) in enumerate(ISUBS):
                    rsum = st.tile([128, 1], F32, tag="rsum")
                    nc.vector.tensor_add(rsum[:isz], ps2[s][1][:isz, 451:452],
                                         invalid[s][:isz])
                    rs = st.tile([128, 1], F32, tag="rs")
                    nc.vector.reciprocal(rs[:isz], rsum[:isz])
                    vt = st.tile([128, OUT], F32, tag="vt", bufs=1)
                    nc.vector.tensor_scalar_mul(vt[:isz, 0:452], ps2[s][0][:isz], rs[:isz])
                    nc.vector.tensor_scalar_mul(vt[:isz, 452:OUT], ps2[s][1][:isz, 0:449], rs[:isz])
                    nc.vector.tensor_add(vt[:isz], vt[:isz], bbc[:isz])
                    xv = per.tile([128, OUT], BF16, tag=f"x{vi}{s}")
                    nc.scalar.activation(xv[:isz], vt[:isz], AF.Relu)
                    xs[(vi, s)] = xv

            # =================== CAM fusion =============================
            y1 = {}
            for br, coff in (("l", 0), ("g", 4)):
                for s, (i0, isz) in enumerate(ISUBS):
                    t = per.tile([128, OUT], F32, tag=f"y1{br}{s}")
                    tmp = st.tile([128, OUT], F32, tag="cl_tmp", bufs=1)
                    nc.vector.tensor_scalar_mul(t[:isz], xs[(0, s)][:isz],
                                                camb[:isz, coff:coff + 1])
                    for c in range(1, 4):
                        nc.vector.tensor_scalar_mul(tmp[:isz], xs[(c, s)][:isz],
                                                    camb[:isz, coff + c: coff + c + 1])
                        nc.vector.tensor_add(t[:isz], t[:isz], tmp[:isz])
                    y1[(br, s)] = t

            def stats_round(srcs, tag):
                # sums over valid rows: cols (S_l, S_g, Q_l, Q_g)
                stp = pss.tile([1, 4], F32, tag="small")
                for s, (i0, isz) in enumerate(ISUBS):
                    sc = st.tile([128, 4], F32, tag="scst", bufs=2)
                    sq = st.tile([128, OUT], F32, tag="sqscr", bufs=1)
                    for bi, br in enumerate(("l", "g")):
                        nc.vector.reduce_sum(sc[:isz, bi: bi + 1],
                                             srcs[(br, s)][:isz],
                                             axis=mybir.AxisListType.X)
                        nc.scalar.activation(sq[:isz], srcs[(br, s)][:isz],
                                             AF.Square,
                                             accum_out=sc[:isz, 2 + bi: 3 + bi])
                    nc.vector.tensor_scalar_mul(sc[:isz], sc[:isz], valid[s][:isz])
                    nc.tensor.matmul(stp[:1], ones[:isz], sc[:isz],
                                     start=(s == 0), stop=(s == 1))
                loc = st.tile([1, 4], F32, tag=f"loc{tag}")
                nc.vector.tensor_copy(loc, stp)
                agi = dr.tile([1, 4], F32, tag=f"sti{tag}")
                ago = dr.tile([NCORES, 4], F32, tag=f"sto{tag}", addr_space="Shared")
                nc.sync.dma_start(agi, loc)
                nc.gpsimd.collective_compute(
                    "AllGather", mybir.AluOpType.bypass, replica_groups=rg,
                    ins=[agi.opt()], outs=[ago.opt()])
                gsb = st.tile([NCORES, 4], F32, tag=f"gsb{tag}")
                nc.sync.dma_start(gsb, ago[:, :])
                gps = pss.tile([1, 4], F32, tag="small")
                nc.tensor.matmul(gps[:1], ones[:NCORES], gsb, start=True, stop=True)
                # -> mean row [1,2] (l,g), var row [1,2]
                mrow = per.tile([1, 4], F32, tag=f"mrow{tag}")
                nc.scalar.mul(mrow, gps, 1.0 / CNT)
                m_ = mrow[0:1, 0:2]
                msq = st.tile([1, 2], F32, tag=f"msq{tag}")
                nc.vector.tensor_mul(msq, m_, m_)
                var = per.tile([1, 2], F32, tag=f"var{tag}")
                nc.vector.tensor_sub(var, mrow[0:1, 2:4], msq)
                return m_, var

            m1, var1 = stats_round(y1, "r1")
            std1 = st.tile([1, 2], F32, tag="std1")
            nc.scalar.activation(std1, var1, AF.Sqrt, bias=epst[0:1, 0:1])
            rs1 = st.tile([1, 2], F32, tag="rs1")
            nc.vector.reciprocal(rs1, std1)
            nmrs1 = st.tile([1, 2], F32, tag="nmrs1")
            nc.vector.tensor_mul(nmrs1, m1, rs1)
            nc.scalar.mul(nmrs1, nmrs1, -1.0)
            pk1 = st.tile([1, 4], F32, tag="pk1")
            nc.vector.tensor_copy(pk1[:, 0:2], rs1)
            nc.vector.tensor_copy(pk1[:, 2:4], nmrs1)
            d1 = dr.tile([1, 4], F32, tag="d1")
            nc.sync.dma_start(d1, pk1)
            r1bc = per.tile([128, 4], F32, tag="r1bc")
            nc.sync.dma_start(r1bc, _bcast(d1, 128, 4))
            # y1r = relu(y1 * rs + (-m*rs))  (in place)
            for bi, br in enumerate(("l", "g")):
                for s, (i0, isz) in enumerate(ISUBS):
                    nc.scalar.activation(y1[(br, s)][:isz], y1[(br, s)][:isz],
                                         AF.Relu, scale=r1bc[:isz, bi:bi + 1],
                                         bias=r1bc[:isz, 2 + bi:3 + bi])

            mr, vr = stats_round(y1, "r2")
            # per-channel alpha_l, alpha_g, beta  [1,4] each
            al = {}
            for bi, (br, coff) in enumerate((("l", 8), ("g", 12))):
                w2 = camb[0:1, coff:coff + 4]
                w2sq = st.tile([1, 4], F32, tag=f"w2sq{br}")
                nc.vector.tensor_mul(w2sq, w2, w2)
                nc.vector.tensor_scalar(w2sq, w2sq, vr[0:1, bi:bi + 1], EPS,
                                        op0=MUL, op1=ADD)
                nc.scalar.activation(w2sq, w2sq, AF.Sqrt)
                nc.vector.reciprocal(w2sq, w2sq)
                a_ = st.tile([1, 4], F32, tag=f"al{br}")
                nc.vector.tensor_mul(a_, w2, w2sq)
                al[br] = a_
            beta = st.tile([1, 4], F32, tag="beta")
            bt = st.tile([1, 4], F32, tag="bt")
            nc.vector.tensor_scalar_mul(beta, al["l"], mr[0:1, 0:1])
            nc.vector.tensor_scalar_mul(bt, al["g"], mr[0:1, 1:2])
            nc.vector.tensor_add(beta, beta, bt)
            nc.scalar.mul(beta, beta, -1.0)
            pk2 = st.tile([1, 12], F32, tag="pk2")
            nc.vector.tensor_copy(pk2[:, 0:4], al["l"])
            nc.vector.tensor_copy(pk2[:, 4:8], al["g"])
            nc.vector.tensor_copy(pk2[:, 8:12], beta)
            d2 = dr.tile([1, 12], F32, tag="d2")
            nc.sync.dma_start(d2, pk2)
            r2bc = per.tile([128, 12], F32, tag="r2bc")
            nc.sync.dma_start(r2bc, _bcast(d2, 128, 12))

            # fuse: acc = sum_c x_c * sigmoid(al_c*u + ag_c*w + beta_c)
            wabc = per.tile([128, 2 * OUT], F32, tag="wabc")
            nc.sync.dma_start(wabc[:, 0:OUT], _bcast(ins["wab"], 128, OUT, offset=0))
            nc.sync.dma_start(wabc[:, OUT:2 * OUT], _bcast(ins["wab"], 128, OUT, offset=OUT))
            for s, (i0, isz) in enumerate(ISUBS):
                acc = per.tile([128, OUT], F32, tag=f"acc{s}")
                zc = st.tile([128, OUT], F32, tag="zc", bufs=1)
                z2 = st.tile([128, OUT], F32, tag="z2", bufs=1)
                for c in range(4):
                    nc.scalar.activation(zc[:isz], y1[("g", s)][:isz], AF.Identity,
                                         scale=r2bc[:isz, 4 + c:5 + c],
                                         bias=r2bc[:isz, 8 + c:9 + c])
                    nc.vector.tensor_scalar_mul(z2[:isz], y1[("l", s)][:isz],
                                                r2bc[:isz, c:c + 1])
                    nc.vector.tensor_add(zc[:isz], zc[:isz], z2[:isz])
                    nc.scalar.activation(zc[:isz], zc[:isz], AF.Sigmoid)
                    if c == 0:
                        nc.vector.tensor_mul(acc[:isz], xs[(c, s)][:isz], zc[:isz])
                    else:
                        nc.vector.tensor_mul(z2[:isz], xs[(c, s)][:isz], zc[:isz])
                        nc.vector.tensor_add(acc[:isz], acc[:isz], z2[:isz])
                qrt = st.tile([128, 2], F32, tag="qrt", bufs=2)
                nc.vector.tensor_mul(zc[:isz], acc[:isz], wabc[:isz, 0:OUT])
                nc.vector.reduce_sum(qrt[:isz, 0:1], zc[:isz], axis=mybir.AxisListType.X)
                nc.vector.tensor_mul(zc[:isz], acc[:isz], wabc[:isz, OUT:2 * OUT])
                nc.vector.reduce_sum(qrt[:isz, 1:2], zc[:isz], axis=mybir.AxisListType.X)
                nc.sync.dma_start(qr_out[i0:i0 + isz, :], qrt[:isz])
    nc.compile()
    return nc


# ======================= host side ==================================

def _rearr(a, nk):
    # [KP, C] -> [128, NK*C] (block kt at cols [kt*C:(kt+1)*C])
    kp, c = a.shape
    return np.ascontiguousarray(
        a.reshape(nk, 128, c).transpose(1, 0, 2).reshape(128, nk * c))


def _prep(inputs):
    bf = ml_dtypes.bfloat16
    per_core = [dict() for _ in range(NCORES)]
    frows = [np.arange(c * CI, (c + 1) * CI) for c in range(NCORES)]
    valids = [(fr < NROWS) for fr in frows]

    for V in VIEWS:
        n, N, off, CJ, NK, KP, JG = (V["name"], V["N"], V["off"], V["CJ"],
                                     V["NK"], V["KP"], V["JG"])
        feat = np.asarray(inputs[f"feat_{n}"], np.float32)
        adj = np.asarray(inputs[f"adj_{n}"])
        W = np.asarray(inputs[f"W_{n}"], np.float64)
        a_src = np.asarray(inputs[f"a_src_{n}"], np.float64)
        a_dst = np.asarray(inputs[f"a_dst_{n}"], np.float64)
        M = (adj != 0).astype(np.float32)
        np.fill_diagonal(M, 1.0)
        WT = W.T  # [N, OUT]
        Wx = np.zeros((KP, HC), np.float32)
        Wx[:N, :OUT] = WT
        Wx[:N, OUT] = WT @ a_src
        Wx[:N, OUT + 1] = WT @ a_dst
        wx_dev = _rearr(Wx, NK).astype(bf)
        featb = feat.astype(bf).astype(np.float32)  # pre-round once
        for c in range(NCORES):
            j0, j1 = c * CJ, min((c + 1) * CJ, N)
            ft = np.zeros((KP, CJ), np.float32)
            ft[:N, :j1 - j0] = featb[j0:j1].T
            per_core[c][f"featT_{n}"] = _rearr(ft, NK).astype(bf)
            fr, va = frows[c], valids[c]
            vrow = np.where(fr < OUT, fr, off + fr - OUT)[va]
            fu = np.zeros((KP, CI), np.float32)
            fu[:N, :vrow.size] = featb[vrow].T
            per_core[c][f"featU_{n}"] = _rearr(fu, NK).astype(bf)
            mt = np.zeros((JG, CI), np.float32)
            mt[:N, :vrow.size] = M[:, vrow]
            per_core[c][f"maskT_{n}"] = mt.astype(bf)
            per_core[c][f"Wx_{n}"] = wx_dev
            per_core[c][f"b_{n}"] = np.asarray(
                inputs[f"b_{n}"], np.float32).reshape(1, OUT)

    # collapsed pair-MLP vector + constant
    mW1 = np.asarray(inputs["mW1"], np.float64)
    mW2 = np.asarray(inputs["mW2"], np.float64)
    mW3 = np.asarray(inputs["mW3"], np.float64)
    mW4 = np.asarray(inputs["mW4"], np.float64)
    w432 = mW4 @ mW3 @ mW2                      # [1,1024]
    wfull = (w432 @ mW1)[0]                     # [1802]
    cconst = (np.asarray(inputs["mb1"], np.float64) @ w432[0]
              + np.asarray(inputs["mb2"], np.float64) @ (mW4 @ mW3)[0]
              + np.asarray(inputs["mb3"], np.float64) @ mW4[0]
              + np.asarray(inputs["mb4"], np.float64)[0])
    wab = np.stack([wfull[:OUT] / 4.0, wfull[OUT:] / 4.0]).astype(np.float32)

    camw = np.concatenate([
        np.asarray(inputs["lw1"], np.float32).ravel(),
        np.asarray(inputs["gw1"], np.float32).ravel(),
        np.asarray(inputs["lw2"], np.float32).ravel(),
        np.asarray(inputs["gw2"], np.float32).ravel()]).reshape(1, 16)

    md = np.asarray(inputs["mirna_disease"], np.float32)
    for c in range(NCORES):
        fr, va = frows[c], valids[c]
        mdc = np.zeros((CI, OUT), np.float32)
        mdc[va] = md[fr[va]]
        per_core[c]["md"] = mdc.astype(bf)
        per_core[c]["validi"] = va.astype(np.float32).reshape(CI, 1)
        per_core[c]["camw"] = camw
        per_core[c]["wab"] = wab
    return per_core, float(cconst)


def kernel(**inputs):
    global LAST_RESULTS
    if "nc" not in _CACHE:
        _CACHE["nc"] = build_graph()
    nc = _CACHE["nc"]
    in_maps, cconst = _prep(inputs)
    res = run_bass_kernel_spmd(nc, in_maps, core_ids=list(range(NCORES)))
    LAST_RESULTS = res
    qr = np.concatenate([np.asarray(res.results[c]["qr"]) for c in range(NCORES)])
    q, r = qr[:NROWS, 0], qr[:NROWS, 1]
    ts = np.asarray(inputs["test_sample"])
    out = (q[ts[:, 0]] + r[ts[:, 1]] + cconst).astype(np.float32)
    return out.reshape(NPAIRS, 1)
